# revision 6
# baseline (speedup 1.0000x reference)
"""Trainium2 Bass kernel for nn_HardLinearAttention.

Math: out = Z + (alpha/n) * P @ Z @ M @ Z.T @ Q @ Z with
  P = e_last e_last^T, M = lower-tri lambda^(i-j) (last row/col zero),
  Q = [[-I, I],[0,0]] blocks.
Because P has a single nonzero (bottom-right), the update is rank-1: only the
last row of the output differs from Z.  With z = Z[-1,:] (masked at col n):
  r[j] = sum_k lambda^k z[j+k]          (geometric window, 128 taps)
  s[i] = sum_j Z[i,j] r[j]   (i < d)    (only s[0:d] survives Q)
  u[j] = sum_k s[k] (Z[d+k,j] - Z[k,j])
  out[-1,:] = Z[-1,:] + (alpha/n) u ;  out[i,:] = Z[i,:] otherwise.

Sharding: context axis (n+1) split 8 ways (1025 cols/core over a zero-padded
8200-wide array).  Each core computes its local r block and its partial
s-row sT = sum_j r[j] ZtopT[j, :] (one wide matmul per j-tile), a 2KB DRAM
AllReduce(add) sums sT across cores, then each core transposes sT back to
column form with 4 tiny matmuls and computes u for its columns.

Performance structure (v4):
  - Bulk copy (rows 0..1023) moves as bf16 (host round-to-nearest both ways;
    ~1.7e-3 rel err, well under the 2e-2 gate), halving DMA-ring traffic.
    The updated last row stays f32.
  - Queues: SP carries only the small critical chain (lam/win/ztp/zlast),
    so no core's collective input is starved behind the bulk backlog; Act
    carries the bulk loads + half the stores; Pool (SWDGE) carries the
    collective DMAs + the other half of the stores.
  - The collective payload is the row-form sT (one 2KB descriptor each way)
    and AllReduce(add) replaces AllGather+local sum.
"""

import sys

for _p in ("/opt/trn_rl_repo", "/root/.axon_site/_ro/trn_rl_repo"):
    if _p not in sys.path:
        sys.path.append(_p)

import ml_dtypes
import numpy as np

import concourse.bacc as bacc
import concourse.bass as bass
import concourse.mybir as mybir
import concourse.tile as tile
from concourse.ap import AP
from concourse import bass_utils

F32 = mybir.dt.float32
BF16 = mybir.dt.bfloat16
NP_BF16 = ml_dtypes.bfloat16

D = 512          # feature dim d
N = 8192         # context length n
R = 2 * D + 1    # 1025 rows
NC = 8           # cores
L = 1025         # columns per core (8 * 1025 = 8200 >= 8193)
WTOT = NC * L    # 8200 padded width
W = 128          # geometric window taps
LPAD = 1152      # local column count padded to 9*128 for full j-tiles
ZWLEN = 1280     # zwin input length: LPAD + W - 1 = 1279, rounded up
NT_J = LPAD // 128   # 9 j-tiles
NT_K = D // 128      # 4 feature tiles
NT_ROW = 8           # full 128-row tiles (rows 0..1023)
J_CHUNKS = [(0, 512), (512, 1024), (1024, 1025)]

_PROGRAM = None


def _build_program():
    nc = bacc.Bacc(
        "TRN2",
        target_bir_lowering=False,
        debug=False,
        enable_asserts=False,
        num_devices=NC,
    )

    zc_d = nc.dram_tensor("zc", [128, NT_ROW, L], BF16, kind="ExternalInput")
    ztp_d = nc.dram_tensor("ztp", [128, NT_J, D], BF16, kind="ExternalInput")
    zlast_d = nc.dram_tensor("zlast", [L], F32, kind="ExternalInput")
    zwin_d = nc.dram_tensor("zwin", [ZWLEN], BF16, kind="ExternalInput")
    lam_d = nc.dram_tensor("lam", [W], BF16, kind="ExternalInput")
    alpha_d = nc.dram_tensor("alpha", [1], F32, kind="ExternalInput")
    out_d = nc.dram_tensor("out", [128, NT_ROW, L], BF16, kind="ExternalOutput")
    outlast_d = nc.dram_tensor("outlast", [L], F32, kind="ExternalOutput")

    with tile.TileContext(nc) as tc:
        with (
            tc.tile_pool(name="consts", bufs=1) as consts,
            tc.tile_pool(name="zbuf", bufs=1) as zbuf,
            tc.tile_pool(name="work", bufs=1) as work,
            tc.tile_pool(name="rp_ps", bufs=2, space=bass.MemorySpace.PSUM) as rp_ps,
            tc.tile_pool(name="st_ps", bufs=1, space=bass.MemorySpace.PSUM) as st_ps,
            tc.tile_pool(name="sc_ps", bufs=2, space=bass.MemorySpace.PSUM) as sc_ps,
            tc.tile_pool(name="u_ps", bufs=2, space=bass.MemorySpace.PSUM) as u_ps,
            tc.tile_pool(name="ccdram", bufs=1, space="DRAM") as ccdram,
        ):
            # ---- SP queue: ONLY the critical small chain ----------------
            lam0 = consts.tile([128, 1], BF16, name="lam0")
            nc.sync.dma_start(lam0[:], lam_d[0:W].unsqueeze(1))

            # overlapping window: win[k, j] = zwin[k + j]
            win = consts.tile([128, LPAD], BF16, name="win")
            nc.sync.dma_start(win[:], AP(zwin_d, 0, [[1, 128], [1, LPAD]]))

            # ZtopT bf16 (9.2KB per-partition descriptors)
            ztp_sb = zbuf.tile([128, NT_J, D], BF16, name="ztp_sb")
            nc.sync.dma_start(ztp_sb[0:64, :, :], ztp_d[0:64, :, :])
            nc.sync.dma_start(ztp_sb[64:128, :, :], ztp_d[64:128, :, :])

            zlast = work.tile([1, L], F32, name="zlast")
            nc.sync.dma_start(zlast[:], zlast_d[:].unsqueeze(0))
            alpha_sb = consts.tile([1, 1], F32, name="alpha_sb")
            nc.sync.dma_start(alpha_sb[:], alpha_d[0:1].unsqueeze(1))

            scale_sb = consts.tile([1, 1], F32, name="scale_sb")
            nc.vector.tensor_scalar_mul(scale_sb[:], alpha_sb[:], 1.0 / float(N))

            # ---- Act queue: bulk loads (16.4KB descs), later stores -----
            zbig = zbuf.tile([128, NT_ROW, L], BF16, name="zbig")
            for q in range(4):
                nc.scalar.dma_start(
                    zbig[q * 32:(q + 1) * 32, :, :],
                    zc_d[q * 32:(q + 1) * 32, :, :],
                )

            # ---- stage 1: r columns  r[jt] = win_tile.T @ lam -----------
            rcols = work.tile([128, NT_J], BF16, name="rcols")
            for t in range(NT_J):
                rp = rp_ps.tile([128, 1], F32, name="rp", tag="rp")
                nc.tensor.matmul(
                    rp[:], win[:, t * 128:(t + 1) * 128], lam0[:],
                    start=True, stop=True,
                )
                nc.vector.tensor_copy(rcols[:, t:t + 1], rp[:])

            # ---- stage 2 (flipped): sT[1, i] = sum_t rcols_t.T @ ztp_t --
            sT = st_ps.tile([1, D], F32, name="sT", tag="sT")
            for t in range(NT_J):
                nc.tensor.matmul(
                    sT[:], rcols[:, t:t + 1], ztp_sb[:, t, :],
                    start=(t == 0), stop=(t == NT_J - 1),
                )
            sT_sb = work.tile([1, D], F32, name="sT_sb")
            nc.vector.tensor_copy(sT_sb[:], sT[:])

            # ---- AllReduce(add) of sT: one 2KB descriptor each way ------
            cc_in = ccdram.tile([1, D], F32, name="cc_in")
            cc_out = ccdram.tile([1, D], F32, name="cc_out")
            nc.gpsimd.dma_start(cc_in[:], sT_sb[:])
            nc.gpsimd.collective_compute(
                "AllReduce",
                mybir.AluOpType.add,
                replica_groups=[list(range(NC))],
                ins=[cc_in.opt()],
                outs=[cc_out.opt()],
            )
            # SWDGE share of the bulk stores (behind the collective trigger)
            nc.gpsimd.dma_start(out_d[0:32, :, :], zbig[0:32, :, :])
            nc.gpsimd.dma_start(out_d[32:64, :, :], zbig[32:64, :, :])
            sT_sum = work.tile([1, D], F32, name="sT_sum")
            nc.gpsimd.dma_start(sT_sum[:], cc_out[:])

            # ---- bulk stores, upper half on the Act queue ---------------
            nc.scalar.dma_start(out_d[64:96, :, :], zbig[64:96, :, :])
            nc.scalar.dma_start(out_d[96:128, :, :], zbig[96:128, :, :])

            # ---- stage 3 prep: zd = Zmid - Ztop (before collective, so
            # the in-order vector engine isn't stalled on the mesh) -------
            zd = []
            for kt in range(NT_K):
                zd_t = work.tile([128, L], BF16, name=f"zd{kt}", tag=f"zd{kt}")
                nc.vector.tensor_sub(zd_t[:], zbig[:, NT_K + kt, :], zbig[:, kt, :])
                zd.append(zd_t)

            # ---- post-collective: transpose sT_sum into column form -----
            sT_sum_bf = work.tile([1, D], BF16, name="sT_sum_bf")
            nc.vector.tensor_copy(sT_sum_bf[:], sT_sum[:])
            ssum_bf = work.tile([128, NT_K], BF16, name="ssum_bf")
            for kt in range(NT_K):
                sp = sc_ps.tile([128, 1], F32, name="sp", tag="sp")
                nc.tensor.matmul(
                    sp[:], sT_sum_bf[0:1, kt * 128:(kt + 1) * 128],
                    lam0[0:1, 0:1],  # lambda^0 == 1 exactly
                    start=True, stop=True,
                )
                nc.vector.tensor_copy(ssum_bf[:, kt:kt + 1], sp[:])

            # ---- stage 3: u = zd.T @ s; last row = zlast + scale*u ------
            for (j0, j1) in J_CHUNKS:
                u = u_ps.tile([1, j1 - j0], F32, name="u", tag="u")
                for kt in range(NT_K):
                    nc.tensor.matmul(
                        u[:], ssum_bf[:, kt:kt + 1], zd[kt][:, j0:j1],
                        start=(kt == 0), stop=(kt == NT_K - 1),
                    )
                newrow = work.tile([1, j1 - j0], F32, name="newrow", tag="newrow")
                nc.vector.scalar_tensor_tensor(
                    newrow[:], u[:], scale_sb[:], zlast[:, j0:j1],
                    op0=mybir.AluOpType.mult, op1=mybir.AluOpType.add,
                )
                nc.sync.dma_start(outlast_d[j0:j1].unsqueeze(0), newrow[:])

    nc.compile()
    return nc


def _get_program():
    global _PROGRAM
    if _PROGRAM is None:
        _PROGRAM = _build_program()
    return _PROGRAM


def _make_in_maps(Z, alpha, M=None):
    Z = np.asarray(Z, dtype=np.float32)
    alpha = np.asarray(alpha, dtype=np.float32).reshape(1)
    # lambda powers; prefer deriving from M's first column when provided.
    if M is not None:
        lam = np.ascontiguousarray(np.asarray(M)[0:W, 0], dtype=np.float32)
    else:
        lam = (0.9 ** np.arange(W)).astype(np.float32)
    lam_bf = lam.astype(NP_BF16)

    Zp = np.zeros((R, WTOT), dtype=np.float32)
    Zp[:, : N + 1] = Z
    zmpad = np.zeros(WTOT + ZWLEN, dtype=np.float32)
    zmpad[:N] = Z[R - 1, :N]  # col n masked to zero (M's last row is zero)

    in_maps = []
    for c in range(NC):
        j0 = c * L
        shard = Zp[:, j0:j0 + L]
        # rows 0..1023 permuted: zc[p, t, :] = shard[t*128 + p, :], bf16
        zc = np.ascontiguousarray(
            shard[:1024].reshape(NT_ROW, 128, L).transpose(1, 0, 2)
        ).astype(NP_BF16)
        # ZtopT padded to 1152 local columns, then permuted the same way
        ztT = np.zeros((LPAD, D), dtype=np.float32)
        ztT[:L] = shard[0:D].T
        ztp = np.ascontiguousarray(
            ztT.reshape(NT_J, 128, D).transpose(1, 0, 2)
        ).astype(NP_BF16)
        in_maps.append(
            {
                "zc": zc,
                "ztp": ztp,
                "zlast": np.ascontiguousarray(shard[R - 1]),
                "zwin": np.ascontiguousarray(zmpad[j0:j0 + ZWLEN]).astype(NP_BF16),
                "lam": lam_bf,
                "alpha": alpha,
            }
        )
    return in_maps


def kernel(Z, alpha, P=None, M=None, Q=None, **_ignored):
    nc = _get_program()
    in_maps = _make_in_maps(Z, alpha, M)
    res = bass_utils.run_bass_kernel_spmd(nc, in_maps, core_ids=list(range(NC)))
    full = np.zeros((R, WTOT), dtype=np.float32)
    for c in range(NC):
        j0 = c * L
        rows = (
            res.results[c]["out"].astype(np.float32)
            .transpose(1, 0, 2).reshape(1024, L)
        )
        full[:1024, j0:j0 + L] = rows
        full[R - 1, j0:j0 + L] = res.results[c]["outlast"]
    return full[:, : N + 1].astype(np.float32)


# revision 8
# speedup vs baseline: 1.9087x; 1.9087x over previous
"""Trainium2 Bass kernel for nn_HardLinearAttention.

Math: out = Z + (alpha/n) * P @ Z @ M @ Z.T @ Q @ Z with
  P = e_last e_last^T, M = lower-tri lambda^(i-j) (last row/col zero),
  Q = [[-I, I],[0,0]] blocks.
Because P has a single nonzero (bottom-right), the update is rank-1: only the
last row of the output differs from Z.  With z = Z[-1,:] (masked at col n):
  r[j] = sum_k lambda^k z[j+k]          (geometric window, 128 taps)
  s[i] = sum_j Z[i,j] r[j]   (i < d)    (only s[0:d] survives Q)
  u[j] = sum_k s[k] (Z[d+k,j] - Z[k,j])
  out[-1,:] = Z[-1,:] + (alpha/n) u ;  out[i,:] = Z[i,:] otherwise.

Sharding: context axis (n+1) split 8 ways (1025 cols/core over a zero-padded
8200-wide array).  Each core computes its local r block and its partial
s-row sT = sum_j r[j] ZtopT[j, :] (one wide matmul per j-tile), a 2KB DRAM
AllReduce(add) sums sT across cores, then each core transposes sT back to
column form with 4 tiny matmuls and computes u for its columns.

Performance structure (v4):
  - Bulk copy (rows 0..1023) moves as bf16 (host round-to-nearest both ways;
    ~1.7e-3 rel err, well under the 2e-2 gate), halving DMA-ring traffic.
    The updated last row stays f32.
  - Queues: SP carries only the small critical chain (lam/win/ztp/zlast),
    so no core's collective input is starved behind the bulk backlog; Act
    carries the bulk loads + half the stores; Pool (SWDGE) carries the
    collective DMAs + the other half of the stores.
  - The collective payload is the row-form sT (one 2KB descriptor each way)
    and AllReduce(add) replaces AllGather+local sum.
"""

import sys

for _p in ("/opt/trn_rl_repo", "/root/.axon_site/_ro/trn_rl_repo"):
    if _p not in sys.path:
        sys.path.append(_p)

import ml_dtypes
import numpy as np

import concourse.bacc as bacc
import concourse.bass as bass
import concourse.mybir as mybir
import concourse.tile as tile
from concourse.ap import AP
from concourse import bass_utils

F32 = mybir.dt.float32
BF16 = mybir.dt.bfloat16
NP_BF16 = ml_dtypes.bfloat16

D = 512          # feature dim d
N = 8192         # context length n
R = 2 * D + 1    # 1025 rows
NC = 8           # cores
L = 1025         # columns per core (8 * 1025 = 8200 >= 8193)
WTOT = NC * L    # 8200 padded width
W = 128          # geometric window taps
LPAD = 1152      # local column count padded to 9*128 for full j-tiles
ZWLEN = 1280     # zwin input length: LPAD + W - 1 = 1279, rounded up
NT_J = LPAD // 128   # 9 j-tiles
NT_K = D // 128      # 4 feature tiles
NT_ROW = 8           # full 128-row tiles (rows 0..1023)
J_CHUNKS = [(0, 512), (512, 1024), (1024, 1025)]

_PROGRAM = None


def _build_program():
    nc = bacc.Bacc(
        "TRN2",
        target_bir_lowering=False,
        debug=False,
        enable_asserts=False,
        num_devices=NC,
    )

    zc_d = nc.dram_tensor("zc", [128, NT_ROW, L], BF16, kind="ExternalInput")
    ztp_d = nc.dram_tensor("ztp", [128, NT_J, D], BF16, kind="ExternalInput")
    zlast_d = nc.dram_tensor("zlast", [L], F32, kind="ExternalInput")
    zwin_d = nc.dram_tensor("zwin", [ZWLEN], BF16, kind="ExternalInput")
    lam_d = nc.dram_tensor("lam", [W], BF16, kind="ExternalInput")
    alpha_d = nc.dram_tensor("alpha", [1], F32, kind="ExternalInput")
    out_d = nc.dram_tensor("out", [128, NT_ROW, L], BF16, kind="ExternalOutput")
    outlast_d = nc.dram_tensor("outlast", [L], F32, kind="ExternalOutput")

    with tile.TileContext(nc) as tc:
        with (
            tc.tile_pool(name="consts", bufs=1) as consts,
            tc.tile_pool(name="zbuf", bufs=1) as zbuf,
            tc.tile_pool(name="work", bufs=1) as work,
            tc.tile_pool(name="rp_ps", bufs=2, space=bass.MemorySpace.PSUM) as rp_ps,
            tc.tile_pool(name="st_ps", bufs=1, space=bass.MemorySpace.PSUM) as st_ps,
            tc.tile_pool(name="sc_ps", bufs=2, space=bass.MemorySpace.PSUM) as sc_ps,
            tc.tile_pool(name="u_ps", bufs=2, space=bass.MemorySpace.PSUM) as u_ps,
            tc.tile_pool(name="ccdram", bufs=1, space="DRAM") as ccdram,
        ):
            # ---- SP queue: ONLY the critical small chain ----------------
            lam0 = consts.tile([128, 1], BF16, name="lam0")
            nc.sync.dma_start(lam0[:], lam_d[0:W].unsqueeze(1))

            # overlapping window: win[k, j] = zwin[k + j]
            win = consts.tile([128, LPAD], BF16, name="win")
            nc.sync.dma_start(win[:], AP(zwin_d, 0, [[1, 128], [1, LPAD]]))

            # ZtopT bf16 (9.2KB per-partition descriptors)
            ztp_sb = zbuf.tile([128, NT_J, D], BF16, name="ztp_sb")
            nc.sync.dma_start(ztp_sb[0:64, :, :], ztp_d[0:64, :, :])
            nc.sync.dma_start(ztp_sb[64:128, :, :], ztp_d[64:128, :, :])

            zlast = work.tile([1, L], F32, name="zlast")
            nc.sync.dma_start(zlast[:], zlast_d[:].unsqueeze(0))
            alpha_sb = consts.tile([1, 1], F32, name="alpha_sb")
            nc.sync.dma_start(alpha_sb[:], alpha_d[0:1].unsqueeze(1))

            scale_sb = consts.tile([1, 1], F32, name="scale_sb")
            nc.vector.tensor_scalar_mul(scale_sb[:], alpha_sb[:], 1.0 / float(N))

            # ---- Act queue: bulk loads (16.4KB descs), later stores -----
            zbig = zbuf.tile([128, NT_ROW, L], BF16, name="zbig")
            for q in range(4):
                nc.scalar.dma_start(
                    zbig[q * 32:(q + 1) * 32, :, :],
                    zc_d[q * 32:(q + 1) * 32, :, :],
                )

            # ---- stage 1: r columns  r[jt] = win_tile.T @ lam -----------
            rcols = work.tile([128, NT_J], BF16, name="rcols")
            for t in range(NT_J):
                rp = rp_ps.tile([128, 1], F32, name="rp", tag="rp")
                nc.tensor.matmul(
                    rp[:], win[:, t * 128:(t + 1) * 128], lam0[:],
                    start=True, stop=True,
                )
                nc.vector.tensor_copy(rcols[:, t:t + 1], rp[:])

            # ---- stage 2 (flipped): sT[1, i] = sum_t rcols_t.T @ ztp_t --
            sT = st_ps.tile([1, D], F32, name="sT", tag="sT")
            for t in range(NT_J):
                nc.tensor.matmul(
                    sT[:], rcols[:, t:t + 1], ztp_sb[:, t, :],
                    start=(t == 0), stop=(t == NT_J - 1),
                )
            sT_sb = work.tile([1, D], F32, name="sT_sb")
            nc.vector.tensor_copy(sT_sb[:], sT[:])

            # ---- AllGather of sT (2KB in, one 16KB gather back) ---------
            cc_in = ccdram.tile([1, D], F32, name="cc_in")
            cc_out = ccdram.tile([NC, D], F32, name="cc_out")
            nc.gpsimd.dma_start(cc_in[:], sT_sb[:])
            nc.gpsimd.collective_compute(
                "AllGather",
                mybir.AluOpType.bypass,
                replica_groups=[list(range(NC))],
                ins=[cc_in.opt()],
                outs=[cc_out.opt()],
            )
            # SWDGE share of the bulk stores (behind the collective trigger)
            nc.gpsimd.dma_start(out_d[0:32, :, :], zbig[0:32, :, :])
            nc.gpsimd.dma_start(out_d[32:64, :, :], zbig[32:64, :, :])
            sTg = work.tile([1, NC * D], F32, name="sTg")
            nc.gpsimd.dma_start(
                sTg[:], cc_out.rearrange("r c -> (r c)").unsqueeze(0)
            )

            # ---- bulk stores, upper half on the Act queue ---------------
            nc.scalar.dma_start(out_d[64:96, :, :], zbig[64:96, :, :])
            nc.scalar.dma_start(out_d[96:128, :, :], zbig[96:128, :, :])

            # ---- stage 3 prep: zd = Zmid - Ztop (before collective, so
            # the in-order vector engine isn't stalled on the mesh) -------
            zd = []
            for kt in range(NT_K):
                zd_t = work.tile([128, L], BF16, name=f"zd{kt}", tag=f"zd{kt}")
                nc.vector.tensor_sub(zd_t[:], zbig[:, NT_K + kt, :], zbig[:, kt, :])
                zd.append(zd_t)

            # ---- post-collective: sum the 8 sT rows, back to columns ----
            sT_sum = work.tile([1, D], F32, name="sT_sum")
            nc.vector.tensor_add(sT_sum[:], sTg[:, 0:D], sTg[:, D:2 * D])
            for r_ in range(2, NC):
                nc.vector.tensor_add(
                    sT_sum[:], sT_sum[:], sTg[:, r_ * D:(r_ + 1) * D]
                )
            sT_sum_bf = work.tile([1, D], BF16, name="sT_sum_bf")
            nc.vector.tensor_copy(sT_sum_bf[:], sT_sum[:])
            ssum_bf = work.tile([128, NT_K], BF16, name="ssum_bf")
            for kt in range(NT_K):
                sp = sc_ps.tile([128, 1], F32, name="sp", tag="sp")
                nc.tensor.matmul(
                    sp[:], sT_sum_bf[0:1, kt * 128:(kt + 1) * 128],
                    lam0[0:1, 0:1],  # lambda^0 == 1 exactly
                    start=True, stop=True,
                )
                nc.vector.tensor_copy(ssum_bf[:, kt:kt + 1], sp[:])

            # ---- stage 3: u = zd.T @ s; last row = zlast + scale*u ------
            for (j0, j1) in J_CHUNKS:
                u = u_ps.tile([1, j1 - j0], F32, name="u", tag="u")
                for kt in range(NT_K):
                    nc.tensor.matmul(
                        u[:], ssum_bf[:, kt:kt + 1], zd[kt][:, j0:j1],
                        start=(kt == 0), stop=(kt == NT_K - 1),
                    )
                newrow = work.tile([1, j1 - j0], F32, name="newrow", tag="newrow")
                nc.vector.scalar_tensor_tensor(
                    newrow[:], u[:], scale_sb[:], zlast[:, j0:j1],
                    op0=mybir.AluOpType.mult, op1=mybir.AluOpType.add,
                )
                nc.sync.dma_start(outlast_d[j0:j1].unsqueeze(0), newrow[:])

    nc.compile()
    return nc


def _get_program():
    global _PROGRAM
    if _PROGRAM is None:
        _PROGRAM = _build_program()
    return _PROGRAM


def _make_in_maps(Z, alpha, M=None):
    Z = np.asarray(Z, dtype=np.float32)
    alpha = np.asarray(alpha, dtype=np.float32).reshape(1)
    # lambda powers; prefer deriving from M's first column when provided.
    if M is not None:
        lam = np.ascontiguousarray(np.asarray(M)[0:W, 0], dtype=np.float32)
    else:
        lam = (0.9 ** np.arange(W)).astype(np.float32)
    lam_bf = lam.astype(NP_BF16)

    Zp = np.zeros((R, WTOT), dtype=np.float32)
    Zp[:, : N + 1] = Z
    zmpad = np.zeros(WTOT + ZWLEN, dtype=np.float32)
    zmpad[:N] = Z[R - 1, :N]  # col n masked to zero (M's last row is zero)

    in_maps = []
    for c in range(NC):
        j0 = c * L
        shard = Zp[:, j0:j0 + L]
        # rows 0..1023 permuted: zc[p, t, :] = shard[t*128 + p, :], bf16
        zc = np.ascontiguousarray(
            shard[:1024].reshape(NT_ROW, 128, L).transpose(1, 0, 2)
        ).astype(NP_BF16)
        # ZtopT padded to 1152 local columns, then permuted the same way
        ztT = np.zeros((LPAD, D), dtype=np.float32)
        ztT[:L] = shard[0:D].T
        ztp = np.ascontiguousarray(
            ztT.reshape(NT_J, 128, D).transpose(1, 0, 2)
        ).astype(NP_BF16)
        in_maps.append(
            {
                "zc": zc,
                "ztp": ztp,
                "zlast": np.ascontiguousarray(shard[R - 1]),
                "zwin": np.ascontiguousarray(zmpad[j0:j0 + ZWLEN]).astype(NP_BF16),
                "lam": lam_bf,
                "alpha": alpha,
            }
        )
    return in_maps


def kernel(Z, alpha, P=None, M=None, Q=None, **_ignored):
    nc = _get_program()
    in_maps = _make_in_maps(Z, alpha, M)
    res = bass_utils.run_bass_kernel_spmd(nc, in_maps, core_ids=list(range(NC)))
    full = np.zeros((R, WTOT), dtype=np.float32)
    for c in range(NC):
        j0 = c * L
        rows = (
            res.results[c]["out"].astype(np.float32)
            .transpose(1, 0, 2).reshape(1024, L)
        )
        full[:1024, j0:j0 + L] = rows
        full[R - 1, j0:j0 + L] = res.results[c]["outlast"]
    return full[:, : N + 1].astype(np.float32)


# revision 14
# speedup vs baseline: 2.0646x; 1.0817x over previous
"""Trainium2 Bass kernel for nn_HardLinearAttention.

Math: out = Z + (alpha/n) * P @ Z @ M @ Z.T @ Q @ Z with
  P = e_last e_last^T, M = lower-tri lambda^(i-j) (last row/col zero),
  Q = [[-I, I],[0,0]] blocks.
Because P has a single nonzero (bottom-right), the update is rank-1: only the
last row of the output differs from Z.  With z = Z[-1,:] (masked at col n):
  r[j] = sum_k lambda^k z[j+k]          (geometric window, 128 taps)
  s[i] = sum_j Z[i,j] r[j]   (i < d)    (only s[0:d] survives Q)
  u[j] = sum_k s[k] (Z[d+k,j] - Z[k,j])
  out[-1,:] = Z[-1,:] + (alpha/n) u ;  out[i,:] = Z[i,:] otherwise.

Sharding: context axis (n+1) split 8 ways (1025 cols/core over a zero-padded
8200-wide array).  Each core computes its local r block and its partial
s-row sT = sum_j r[j] ZtopT[j, :] (one wide matmul per j-tile), a 2KB DRAM
AllReduce(add) sums sT across cores, then each core transposes sT back to
column form with 4 tiny matmuls and computes u for its columns.

Performance structure (v4):
  - Bulk copy (rows 0..1023) moves as bf16 (host round-to-nearest both ways;
    ~1.7e-3 rel err, well under the 2e-2 gate), halving DMA-ring traffic.
    The updated last row stays f32.
  - Queues: SP carries only the small critical chain (lam/win/ztp/zlast),
    so no core's collective input is starved behind the bulk backlog; Act
    carries the bulk loads + half the stores; Pool (SWDGE) carries the
    collective DMAs + the other half of the stores.
  - The collective payload is the row-form sT (one 2KB descriptor each way)
    and AllReduce(add) replaces AllGather+local sum.
"""

import sys

for _p in ("/opt/trn_rl_repo", "/root/.axon_site/_ro/trn_rl_repo"):
    if _p not in sys.path:
        sys.path.append(_p)

import ml_dtypes
import numpy as np

import concourse.bacc as bacc
import concourse.bass as bass
import concourse.mybir as mybir
import concourse.tile as tile
from concourse.ap import AP
from concourse import bass_utils

F32 = mybir.dt.float32
BF16 = mybir.dt.bfloat16
NP_BF16 = ml_dtypes.bfloat16

D = 512          # feature dim d
N = 8192         # context length n
R = 2 * D + 1    # 1025 rows
NC = 8           # cores
L = 1025         # columns per core (8 * 1025 = 8200 >= 8193)
WTOT = NC * L    # 8200 padded width
W = 128          # geometric window taps
LPAD = 1152      # local column count padded to 9*128 for full j-tiles
ZWLEN = 1280     # zwin input length: LPAD + W - 1 = 1279, rounded up
NT_J = LPAD // 128   # 9 j-tiles
NT_K = D // 128      # 4 feature tiles
NT_ROW = 8           # full 128-row tiles (rows 0..1023)
J_CHUNKS = [(0, 512), (512, 1024), (1024, 1025)]

_PROGRAM = None


def _build_program():
    nc = bacc.Bacc(
        "TRN2",
        target_bir_lowering=False,
        debug=False,
        enable_asserts=False,
        num_devices=NC,
    )

    zc_d = nc.dram_tensor("zc", [128, NT_ROW, L], BF16, kind="ExternalInput")
    ztp_d = nc.dram_tensor("ztp", [128, NT_J, D], BF16, kind="ExternalInput")
    zlast_d = nc.dram_tensor("zlast", [L], F32, kind="ExternalInput")
    zwin_d = nc.dram_tensor("zwin", [ZWLEN], BF16, kind="ExternalInput")
    lam_d = nc.dram_tensor("lam", [W], BF16, kind="ExternalInput")
    alpha_d = nc.dram_tensor("alpha", [1], F32, kind="ExternalInput")
    ones_d = nc.dram_tensor("ones8", [NC], F32, kind="ExternalInput")
    out_d = nc.dram_tensor("out", [128, NT_ROW, L], BF16, kind="ExternalOutput")
    outlast_d = nc.dram_tensor("outlast", [L], F32, kind="ExternalOutput")

    with tile.TileContext(nc) as tc:
        with (
            tc.tile_pool(name="consts", bufs=1) as consts,
            tc.tile_pool(name="zbuf", bufs=1) as zbuf,
            tc.tile_pool(name="work", bufs=1) as work,
            tc.tile_pool(name="rp_ps", bufs=2, space=bass.MemorySpace.PSUM) as rp_ps,
            tc.tile_pool(name="st_ps", bufs=1, space=bass.MemorySpace.PSUM) as st_ps,
            tc.tile_pool(name="sc_ps", bufs=2, space=bass.MemorySpace.PSUM) as sc_ps,
            tc.tile_pool(name="u_ps", bufs=2, space=bass.MemorySpace.PSUM) as u_ps,
            tc.tile_pool(name="ccdram", bufs=1, space="DRAM") as ccdram,
        ):
            # ---- SP queue: ONLY the critical small chain ----------------
            lam0 = consts.tile([128, 1], BF16, name="lam0")
            nc.sync.dma_start(lam0[:], lam_d[0:W].unsqueeze(1))

            # overlapping window: win[k, j] = zwin[k + j]
            win = consts.tile([128, LPAD], BF16, name="win")
            nc.sync.dma_start(win[:], AP(zwin_d, 0, [[1, 128], [1, LPAD]]))

            # ZtopT bf16 (9.2KB per-partition descriptors)
            ztp_sb = zbuf.tile([128, NT_J, D], BF16, name="ztp_sb")
            nc.sync.dma_start(ztp_sb[0:64, :, :], ztp_d[0:64, :, :])
            nc.sync.dma_start(ztp_sb[64:128, :, :], ztp_d[64:128, :, :])

            zlast = work.tile([1, L], F32, name="zlast")
            nc.sync.dma_start(zlast[:], zlast_d[:].unsqueeze(0))
            alpha_sb = consts.tile([1, 1], F32, name="alpha_sb")
            nc.sync.dma_start(alpha_sb[:], alpha_d[0:1].unsqueeze(1))
            ones8 = consts.tile([NC, 1], F32, name="ones8")
            nc.sync.dma_start(ones8[:], ones_d[:].unsqueeze(1))

            scale_sb = consts.tile([1, 1], F32, name="scale_sb")
            nc.vector.tensor_scalar_mul(scale_sb[:], alpha_sb[:], 1.0 / float(N))

            # ---- Act queue: gate the bulk flood behind the small chain,
            # so no core's collective input is starved on the shared rings
            gate_dr = ccdram.tile([2, 2], BF16, name="gate_dr")
            nc.scalar.dma_start(gate_dr[0:1, :], ztp_sb[63:64, NT_J - 1, 0:2])
            nc.scalar.dma_start(gate_dr[1:2, :], ztp_sb[127:128, NT_J - 1, 0:2])

            # bulk loads (16.4KB descs), later stores
            zbig = zbuf.tile([128, NT_ROW, L], BF16, name="zbig")
            for q in range(4):
                nc.scalar.dma_start(
                    zbig[q * 32:(q + 1) * 32, :, :],
                    zc_d[q * 32:(q + 1) * 32, :, :],
                )

            # ---- stage 1: r columns  r[jt] = win_tile.T @ lam -----------
            rcols = work.tile([128, NT_J], BF16, name="rcols")
            for t in range(NT_J):
                rp = rp_ps.tile([128, 1], F32, name="rp", tag="rp")
                nc.tensor.matmul(
                    rp[:], win[:, t * 128:(t + 1) * 128], lam0[:],
                    start=True, stop=True,
                )
                nc.vector.tensor_copy(rcols[:, t:t + 1], rp[:])

            # ---- stage 2 (flipped): sT[1, i] = sum_t rcols_t.T @ ztp_t --
            sT = st_ps.tile([1, D], F32, name="sT", tag="sT")
            for t in range(NT_J):
                nc.tensor.matmul(
                    sT[:], rcols[:, t:t + 1], ztp_sb[:, t, :],
                    start=(t == 0), stop=(t == NT_J - 1),
                )
            sT_sb = work.tile([1, D], F32, name="sT_sb")
            nc.vector.tensor_copy(sT_sb[:], sT[:])

            # ---- AllGather of sT (2KB in, one 16KB gather back) ---------
            cc_in = ccdram.tile([1, D], F32, name="cc_in")
            cc_out = ccdram.tile([NC, D], F32, name="cc_out")
            nc.gpsimd.dma_start(cc_in[:], sT_sb[:])
            nc.gpsimd.collective_compute(
                "AllGather",
                mybir.AluOpType.bypass,
                replica_groups=[list(range(NC))],
                ins=[cc_in.opt()],
                outs=[cc_out.opt()],
            )
            # SWDGE share of the bulk stores (behind the collective trigger)
            nc.gpsimd.dma_start(out_d[0:32, :, :], zbig[0:32, :, :])
            nc.gpsimd.dma_start(out_d[32:64, :, :], zbig[32:64, :, :])
            sTg = work.tile([NC, D], F32, name="sTg")
            nc.gpsimd.dma_start(sTg[:], cc_out[:, :])

            # ---- bulk stores, upper half on the Act queue ---------------
            nc.scalar.dma_start(out_d[64:96, :, :], zbig[64:96, :, :])
            nc.scalar.dma_start(out_d[96:128, :, :], zbig[96:128, :, :])

            # ---- stage 3 prep: zd = Zmid - Ztop (before collective, so
            # the in-order vector engine isn't stalled on the mesh) -------
            zd = []
            for kt in range(NT_K):
                zd_t = work.tile([128, L], BF16, name=f"zd{kt}", tag=f"zd{kt}")
                nc.vector.tensor_sub(zd_t[:], zbig[:, NT_K + kt, :], zbig[:, kt, :])
                zd.append(zd_t)

            # ---- post-collective: one reduce-transpose matmul per k-tile:
            # s_cols[kt] = sTg[:, kt-block].T @ ones8  (sums the 8 cores
            # and transposes back to column form in a single PE op)
            ssum_bf = work.tile([128, NT_K], BF16, name="ssum_bf")
            for kt in range(NT_K):
                sp = sc_ps.tile([128, 1], F32, name="sp", tag="sp")
                nc.tensor.matmul(
                    sp[:], sTg[:, kt * 128:(kt + 1) * 128], ones8[:],
                    start=True, stop=True,
                )
                nc.vector.tensor_copy(ssum_bf[:, kt:kt + 1], sp[:])

            # ---- stage 3: u = zd.T @ s; last row = zlast + scale*u ------
            for (j0, j1) in J_CHUNKS:
                u = u_ps.tile([1, j1 - j0], F32, name="u", tag="u")
                for kt in range(NT_K):
                    nc.tensor.matmul(
                        u[:], ssum_bf[:, kt:kt + 1], zd[kt][:, j0:j1],
                        start=(kt == 0), stop=(kt == NT_K - 1),
                    )
                newrow = work.tile([1, j1 - j0], F32, name="newrow", tag="newrow")
                nc.vector.scalar_tensor_tensor(
                    newrow[:], u[:], scale_sb[:], zlast[:, j0:j1],
                    op0=mybir.AluOpType.mult, op1=mybir.AluOpType.add,
                )
                nc.sync.dma_start(outlast_d[j0:j1].unsqueeze(0), newrow[:])

    nc.compile()
    return nc


def _get_program():
    global _PROGRAM
    if _PROGRAM is None:
        _PROGRAM = _build_program()
    return _PROGRAM


def _make_in_maps(Z, alpha, M=None):
    Z = np.asarray(Z, dtype=np.float32)
    alpha = np.asarray(alpha, dtype=np.float32).reshape(1)
    # lambda powers; prefer deriving from M's first column when provided.
    if M is not None:
        lam = np.ascontiguousarray(np.asarray(M)[0:W, 0], dtype=np.float32)
    else:
        lam = (0.9 ** np.arange(W)).astype(np.float32)
    lam_bf = lam.astype(NP_BF16)

    Zp = np.zeros((R, WTOT), dtype=np.float32)
    Zp[:, : N + 1] = Z
    zmpad = np.zeros(WTOT + ZWLEN, dtype=np.float32)
    zmpad[:N] = Z[R - 1, :N]  # col n masked to zero (M's last row is zero)

    in_maps = []
    for c in range(NC):
        j0 = c * L
        shard = Zp[:, j0:j0 + L]
        # rows 0..1023 permuted: zc[p, t, :] = shard[t*128 + p, :], bf16
        zc = np.ascontiguousarray(
            shard[:1024].reshape(NT_ROW, 128, L).transpose(1, 0, 2)
        ).astype(NP_BF16)
        # ZtopT padded to 1152 local columns, then permuted the same way
        ztT = np.zeros((LPAD, D), dtype=np.float32)
        ztT[:L] = shard[0:D].T
        ztp = np.ascontiguousarray(
            ztT.reshape(NT_J, 128, D).transpose(1, 0, 2)
        ).astype(NP_BF16)
        in_maps.append(
            {
                "zc": zc,
                "ztp": ztp,
                "zlast": np.ascontiguousarray(shard[R - 1]),
                "zwin": np.ascontiguousarray(zmpad[j0:j0 + ZWLEN]).astype(NP_BF16),
                "lam": lam_bf,
                "alpha": alpha,
                "ones8": np.ones(NC, dtype=np.float32),
            }
        )
    return in_maps


def kernel(Z, alpha, P=None, M=None, Q=None, **_ignored):
    nc = _get_program()
    in_maps = _make_in_maps(Z, alpha, M)
    res = bass_utils.run_bass_kernel_spmd(nc, in_maps, core_ids=list(range(NC)))
    full = np.zeros((R, WTOT), dtype=np.float32)
    for c in range(NC):
        j0 = c * L
        rows = (
            res.results[c]["out"].astype(np.float32)
            .transpose(1, 0, 2).reshape(1024, L)
        )
        full[:1024, j0:j0 + L] = rows
        full[R - 1, j0:j0 + L] = res.results[c]["outlast"]
    return full[:, : N + 1].astype(np.float32)


# revision 16
# speedup vs baseline: 2.1668x; 1.0495x over previous
"""Trainium2 Bass kernel for nn_HardLinearAttention.

Math: out = Z + (alpha/n) * P @ Z @ M @ Z.T @ Q @ Z with
  P = e_last e_last^T, M = lower-tri lambda^(i-j) (last row/col zero),
  Q = [[-I, I],[0,0]] blocks.
Because P has a single nonzero (bottom-right), the update is rank-1: only the
last row of the output differs from Z.  With z = Z[-1,:] (masked at col n):
  r[j] = sum_k lambda^k z[j+k]          (geometric window, 128 taps)
  s[i] = sum_j Z[i,j] r[j]   (i < d)    (only s[0:d] survives Q)
  u[j] = sum_k s[k] (Z[d+k,j] - Z[k,j])
  out[-1,:] = Z[-1,:] + (alpha/n) u ;  out[i,:] = Z[i,:] otherwise.

Sharding: context axis (n+1) split 8 ways (1025 cols/core over a zero-padded
8200-wide array).  Each core computes its local r block and its partial
s-row sT = sum_j r[j] ZtopT[j, :] (one wide matmul per j-tile), a 2KB DRAM
AllReduce(add) sums sT across cores, then each core transposes sT back to
column form with 4 tiny matmuls and computes u for its columns.

Performance structure (v4):
  - Bulk copy (rows 0..1023) moves as bf16 (host round-to-nearest both ways;
    ~1.7e-3 rel err, well under the 2e-2 gate), halving DMA-ring traffic.
    The updated last row stays f32.
  - Queues: SP carries only the small critical chain (lam/win/ztp/zlast),
    so no core's collective input is starved behind the bulk backlog; Act
    carries the bulk loads + half the stores; Pool (SWDGE) carries the
    collective DMAs + the other half of the stores.
  - The collective payload is the row-form sT (one 2KB descriptor each way)
    and AllReduce(add) replaces AllGather+local sum.
"""

import sys

for _p in ("/opt/trn_rl_repo", "/root/.axon_site/_ro/trn_rl_repo"):
    if _p not in sys.path:
        sys.path.append(_p)

import ml_dtypes
import numpy as np

import concourse.bacc as bacc
import concourse.bass as bass
import concourse.mybir as mybir
import concourse.tile as tile
from concourse.ap import AP
from concourse import bass_utils

F32 = mybir.dt.float32
BF16 = mybir.dt.bfloat16
NP_BF16 = ml_dtypes.bfloat16

D = 512          # feature dim d
N = 8192         # context length n
R = 2 * D + 1    # 1025 rows
NC = 8           # cores
L = 1025         # columns per core (8 * 1025 = 8200 >= 8193)
WTOT = NC * L    # 8200 padded width
W = 128          # geometric window taps
LPAD = 1152      # local column count padded to 9*128 for full j-tiles
ZWLEN = 1280     # zwin input length: LPAD + W - 1 = 1279, rounded up
NT_J = LPAD // 128   # 9 j-tiles
NT_K = D // 128      # 4 feature tiles
NT_ROW = 8           # full 128-row tiles (rows 0..1023)
J_CHUNKS = [(0, 512), (512, 1024), (1024, 1025)]

_PROGRAM = None


def _build_program():
    nc = bacc.Bacc(
        "TRN2",
        target_bir_lowering=False,
        debug=False,
        enable_asserts=False,
        num_devices=NC,
    )

    zc_d = nc.dram_tensor("zc", [128, NT_ROW, L], BF16, kind="ExternalInput")
    ztp_d = nc.dram_tensor("ztp", [128, NT_J, D], BF16, kind="ExternalInput")
    zlast_d = nc.dram_tensor("zlast", [L], F32, kind="ExternalInput")
    zwin_d = nc.dram_tensor("zwin", [ZWLEN], BF16, kind="ExternalInput")
    lam_d = nc.dram_tensor("lam", [W], BF16, kind="ExternalInput")
    alpha_d = nc.dram_tensor("alpha", [1], F32, kind="ExternalInput")
    ones_d = nc.dram_tensor("ones8", [NC], F32, kind="ExternalInput")
    out_d = nc.dram_tensor("out", [128, NT_ROW, L], BF16, kind="ExternalOutput")
    outlast_d = nc.dram_tensor("outlast", [L], F32, kind="ExternalOutput")

    with tile.TileContext(nc) as tc:
        with (
            tc.tile_pool(name="consts", bufs=1) as consts,
            tc.tile_pool(name="zbuf", bufs=1) as zbuf,
            tc.tile_pool(name="work", bufs=1) as work,
            tc.tile_pool(name="rp_ps", bufs=2, space=bass.MemorySpace.PSUM) as rp_ps,
            tc.tile_pool(name="st_ps", bufs=1, space=bass.MemorySpace.PSUM) as st_ps,
            tc.tile_pool(name="sc_ps", bufs=2, space=bass.MemorySpace.PSUM) as sc_ps,
            tc.tile_pool(name="u_ps", bufs=2, space=bass.MemorySpace.PSUM) as u_ps,
            tc.tile_pool(name="ccdram", bufs=1, space="DRAM") as ccdram,
        ):
            # ---- SP queue: ONLY the critical small chain ----------------
            lam0 = consts.tile([128, 1], BF16, name="lam0")
            nc.sync.dma_start(lam0[:], lam_d[0:W].unsqueeze(1))

            # overlapping window: win[k, j] = zwin[k + j]
            win = consts.tile([128, LPAD], BF16, name="win")
            nc.sync.dma_start(win[:], AP(zwin_d, 0, [[1, 128], [1, LPAD]]))

            # ZtopT bf16 (9.2KB per-partition descriptors)
            ztp_sb = zbuf.tile([128, NT_J, D], BF16, name="ztp_sb")
            nc.sync.dma_start(ztp_sb[0:64, :, :], ztp_d[0:64, :, :])
            nc.sync.dma_start(ztp_sb[64:128, :, :], ztp_d[64:128, :, :])

            zlast = work.tile([1, L], F32, name="zlast")
            nc.sync.dma_start(zlast[:], zlast_d[:].unsqueeze(0))
            alpha_sb = consts.tile([1, 1], F32, name="alpha_sb")
            nc.sync.dma_start(alpha_sb[:], alpha_d[0:1].unsqueeze(1))
            ones8 = consts.tile([NC, 1], F32, name="ones8")
            nc.sync.dma_start(ones8[:], ones_d[:].unsqueeze(1))

            scale_sb = consts.tile([1, 1], F32, name="scale_sb")
            nc.vector.tensor_scalar_mul(scale_sb[:], alpha_sb[:], 1.0 / float(N))

            # ---- Act queue: gate the bulk flood behind the small chain,
            # so no core's collective input is starved on the shared rings
            gate_dr = ccdram.tile([2, 2], BF16, name="gate_dr")
            nc.scalar.dma_start(gate_dr[0:1, :], ztp_sb[63:64, NT_J - 1, 0:2])
            nc.scalar.dma_start(gate_dr[1:2, :], ztp_sb[127:128, NT_J - 1, 0:2])

            # bulk loads (16.4KB descs), later stores
            zbig = zbuf.tile([128, NT_ROW, L], BF16, name="zbig")
            for q in range(4):
                nc.scalar.dma_start(
                    zbig[q * 32:(q + 1) * 32, :, :],
                    zc_d[q * 32:(q + 1) * 32, :, :],
                )

            # ---- stage 1: r columns  r[jt] = win_tile.T @ lam -----------
            rcols = work.tile([128, NT_J], BF16, name="rcols")
            for t in range(NT_J):
                rp = rp_ps.tile([128, 1], F32, name="rp", tag="rp")
                nc.tensor.matmul(
                    rp[:], win[:, t * 128:(t + 1) * 128], lam0[:],
                    start=True, stop=True,
                )
                nc.vector.tensor_copy(rcols[:, t:t + 1], rp[:])

            # ---- stage 2 (flipped): sT[1, i] = sum_t rcols_t.T @ ztp_t --
            sT = st_ps.tile([1, D], F32, name="sT", tag="sT")
            for t in range(NT_J):
                nc.tensor.matmul(
                    sT[:], rcols[:, t:t + 1], ztp_sb[:, t, :],
                    start=(t == 0), stop=(t == NT_J - 1),
                )
            sT_sb = work.tile([1, D], F32, name="sT_sb")
            nc.vector.tensor_copy(sT_sb[:], sT[:])

            # ---- AllGather of sT (2KB in, one 16KB gather back) ---------
            cc_in = ccdram.tile([1, D], F32, name="cc_in")
            cc_out = ccdram.tile([NC, D], F32, name="cc_out")
            nc.gpsimd.dma_start(cc_in[:], sT_sb[:])
            nc.gpsimd.collective_compute(
                "AllGather",
                mybir.AluOpType.bypass,
                replica_groups=[list(range(NC))],
                ins=[cc_in.opt()],
                outs=[cc_out.opt()],
            )
            # SWDGE share of the bulk stores (behind the collective trigger)
            nc.gpsimd.dma_start(out_d[0:32, :, :], zbig[0:32, :, :])
            nc.gpsimd.dma_start(out_d[32:64, :, :], zbig[32:64, :, :])
            sTg = work.tile([NC, D], F32, name="sTg")
            nc.gpsimd.dma_start(sTg[:], cc_out[:, :])

            # ---- bulk stores, upper half on the Act queue ---------------
            nc.scalar.dma_start(out_d[64:96, :, :], zbig[64:96, :, :])
            nc.scalar.dma_start(out_d[96:128, :, :], zbig[96:128, :, :])

            # ---- stage 3 prep: zd = Zmid - Ztop (before collective, so
            # the in-order vector engine isn't stalled on the mesh) -------
            zd = []
            for kt in range(NT_K):
                zd_t = work.tile([128, L], BF16, name=f"zd{kt}", tag=f"zd{kt}")
                nc.vector.tensor_sub(zd_t[:], zbig[:, NT_K + kt, :], zbig[:, kt, :])
                zd.append(zd_t)

            # ---- post-collective: one reduce-transpose matmul per k-tile:
            # s_cols[kt] = sTg[:, kt-block].T @ ones8  (sums the 8 cores
            # and transposes back to column form in a single PE op)
            ssum_bf = work.tile([128, NT_K], BF16, name="ssum_bf")
            for kt in range(NT_K):
                sp = sc_ps.tile([128, 1], F32, name="sp", tag="sp")
                nc.tensor.matmul(
                    sp[:], sTg[:, kt * 128:(kt + 1) * 128], ones8[:],
                    start=True, stop=True,
                )
                nc.vector.tensor_copy(ssum_bf[:, kt:kt + 1], sp[:])

            # ---- stage 3: u = zd.T @ s; last row = zlast + scale*u ------
            for (j0, j1) in J_CHUNKS:
                u = u_ps.tile([1, j1 - j0], F32, name="u", tag="u")
                for kt in range(NT_K):
                    nc.tensor.matmul(
                        u[:], ssum_bf[:, kt:kt + 1], zd[kt][:, j0:j1],
                        start=(kt == 0), stop=(kt == NT_K - 1),
                    )
                newrow = work.tile([1, j1 - j0], F32, name="newrow", tag="newrow")
                nc.vector.scalar_tensor_tensor(
                    newrow[:], u[:], scale_sb[:], zlast[:, j0:j1],
                    op0=mybir.AluOpType.mult, op1=mybir.AluOpType.add,
                )
                nc.sync.dma_start(outlast_d[j0:j1].unsqueeze(0), newrow[:])

    nc.compile()
    return nc


def _get_program():
    global _PROGRAM
    if _PROGRAM is None:
        _PROGRAM = _build_program()
    return _PROGRAM


def _make_in_maps(Z, alpha, M=None):
    Z = np.asarray(Z, dtype=np.float32)
    alpha = np.asarray(alpha, dtype=np.float32).reshape(1)
    # lambda powers; prefer deriving from M's first column when provided.
    if M is not None:
        lam = np.ascontiguousarray(np.asarray(M)[0:W, 0], dtype=np.float32)
    else:
        lam = (0.9 ** np.arange(W)).astype(np.float32)
    lam_bf = lam.astype(NP_BF16)

    Zp = np.zeros((R, WTOT), dtype=np.float32)
    Zp[:, : N + 1] = Z
    zmpad = np.zeros(WTOT + ZWLEN, dtype=np.float32)
    zmpad[:N] = Z[R - 1, :N]  # col n masked to zero (M's last row is zero)

    in_maps = []
    for c in range(NC):
        j0 = c * L
        shard = Zp[:, j0:j0 + L]
        # rows 0..1023 permuted: zc[p, t, :] = shard[t*128 + p, :], bf16
        zc = np.ascontiguousarray(
            shard[:1024].reshape(NT_ROW, 128, L).transpose(1, 0, 2)
        ).astype(NP_BF16)
        # ZtopT padded to 1152 local columns, then permuted the same way
        ztT = np.zeros((LPAD, D), dtype=np.float32)
        ztT[:L] = shard[0:D].T
        ztp = np.ascontiguousarray(
            ztT.reshape(NT_J, 128, D).transpose(1, 0, 2)
        ).astype(NP_BF16)
        in_maps.append(
            {
                "zc": zc,
                "ztp": ztp,
                "zlast": np.ascontiguousarray(shard[R - 1]),
                "zwin": np.ascontiguousarray(zmpad[j0:j0 + ZWLEN]).astype(NP_BF16),
                "lam": lam_bf,
                "alpha": alpha,
                "ones8": np.ones(NC, dtype=np.float32),
            }
        )
    return in_maps


def kernel(Z, alpha, P=None, M=None, Q=None, **_ignored):
    nc = _get_program()
    in_maps = _make_in_maps(Z, alpha, M)
    res = bass_utils.run_bass_kernel_spmd(nc, in_maps, core_ids=list(range(NC)))
    full = np.zeros((R, WTOT), dtype=np.float32)
    for c in range(NC):
        j0 = c * L
        rows = (
            res.results[c]["out"].astype(np.float32)
            .transpose(1, 0, 2).reshape(1024, L)
        )
        full[:1024, j0:j0 + L] = rows
        full[R - 1, j0:j0 + L] = res.results[c]["outlast"]
    return full[:, : N + 1].astype(np.float32)
